# revision 1
# baseline (speedup 1.0000x reference)
"""BitLinear (4-bit activation quant + ternary weight) Trainium2 kernel.

Full computation:
    xq  = round(clip(x / max_abs(x, row) * 7)) * max_abs / 7      (per-row 4-bit quant)
    wq  = sign_thresholded(w) * mean_abs(w, row)                   (ternary weight)
    out = xq @ wq.T + bias

Strategy (8 NeuronCores, data-parallel over rows of x):
  - Shard x rows 8 ways; replicate weight.
  - On-chip, the matmul runs on exact small integers in bf16 (q in [-7,7],
    sign in {-1,0,1}) so the PE accumulation is exact; the row scale
    (max_abs/7) and column scale (alpha) are applied to the f32 PSUM output
    in one fused scalar_tensor_tensor eviction.
  - Rounding uses the +1.5*2^23 magic-number trick == round-half-even,
    matching jnp.round bit-for-bit.
"""

import os
import sys

os.environ.setdefault("MYCRO_LOCAL_CACHE", "1")

for _p in ("/opt/trn_rl_repo", "/root/.axon_site/_ro/trn_rl_repo"):
    if os.path.isdir(_p) and _p not in sys.path:
        sys.path.insert(0, _p)

import numpy as np

N_CORES = 8
S_SHARD = 4096  # rows of x per core (8*4096 total / 8 cores)
IN_F = 1024
OUT_F = 1024
P = 128  # partitions
N_STILES = S_SHARD // P  # 32
N_KTILES = IN_F // P  # 8
N_OTILES = OUT_F // P  # 8
MM_N = 512  # matmul moving free dim (one PSUM bank of f32)
N_OHALF = OUT_F // MM_N  # 2

MAGIC = 12582912.0  # 1.5 * 2**23: float32 add == round-to-nearest-even
EPS = 1e-06

_prog_cache = {}


def _build_program(with_bias: bool, ablate: str = "full"):
    import concourse.bass as bass
    import concourse.mybir as mybir
    import concourse.tile as tile
    from concourse import bacc, bass_isa
    from concourse.masks import make_identity

    f32 = mybir.dt.float32
    bf16 = mybir.dt.bfloat16
    f8 = mybir.dt.float8e4
    Alu = mybir.AluOpType
    Act = mybir.ActivationFunctionType

    nc = bacc.Bacc("TRN2", target_bir_lowering=False, debug=False)

    x_in = nc.dram_tensor("x_shard", [S_SHARD, IN_F], f32, kind="ExternalInput")
    w_in = nc.dram_tensor("weight", [OUT_F, IN_F], f32, kind="ExternalInput")
    if with_bias:
        b_in = nc.dram_tensor("bias", [OUT_F], f32, kind="ExternalInput")
    out_d = nc.dram_tensor("out", [S_SHARD, OUT_F], f32, kind="ExternalOutput")

    with tile.TileContext(nc) as tc:
        from contextlib import ExitStack as _ES

        _wstack = _ES()
        with (
            tc.tile_pool(name="singles", bufs=1) as singles,
            tc.tile_pool(name="wtmp", bufs=2) as wtmp,
            tc.tile_pool(name="signp", bufs=2) as signp,
            tc.tile_pool(name="xp", bufs=6 if not with_bias else 5) as xp,
            tc.tile_pool(name="tp", bufs=3) as tp,
            tc.tile_pool(name="qtp", bufs=N_STILES + 1) as qtp,
            tc.tile_pool(name="outp", bufs=6 if not with_bias else 5) as outp,
            tc.tile_pool(name="stats", bufs=8) as stats,
            tc.tile_pool(name="ma7p", bufs=N_STILES + 1) as ma7p,
            tc.tile_pool(name="tpsum", bufs=2, space="PSUM") as tpsum,
            tc.tile_pool(name="mpsum", bufs=2, space="PSUM") as mpsum,
            tc.tile_pool(name="dramp", bufs=1, space="DRAM") as dramp,
        ):
            # ---------------- one-time setup ----------------
            identity = singles.tile([P, P], bf16)
            make_identity(nc, identity)
            identity_f = singles.tile([P, P], f32)
            make_identity(nc, identity_f)

            magneg = singles.tile([P, 1], f32)
            nc.vector.memset(magneg, -MAGIC)
            zerob = singles.tile([P, 1], f32)
            nc.vector.memset(zerob, 0.0)

            # signT8[i_sub, k, o] = ternarized sign of weight[o, k*128+i_sub]
            # fp8 for DoubleRow matmuls (values {-1,0,1}: exact)
            signT8 = singles.tile([P, N_KTILES, OUT_F], f8)
            alpha_raw = singles.tile([P, N_OTILES], f32)  # row sums of |w|

            wpool = _wstack.enter_context(tc.tile_pool(name="wpool", bufs=8))
            w_tiles = []
            for j in range(N_OTILES):
                w_t = wpool.tile([P, IN_F], f32, tag="w")
                w_tiles.append(w_t)
                # odd tiles ride the scalar ring immediately; even tiles are
                # issued inside the prologue, interleaved behind the first x
                # loads so the SP ring serves the quant pipeline first
                if j % 2:
                    nc.scalar.dma_start(out=w_t, in_=w_in[j * P : (j + 1) * P, :])

            def emit_wload_even(js):
                for j in js:
                    if j < N_OTILES:
                        nc.sync.dma_start(
                            out=w_tiles[j], in_=w_in[j * P : (j + 1) * P, :]
                        )

            def emit_wabs(j):
                # |w| row sums on DVE, interleaved into the quant stream
                nc.vector.tensor_reduce(
                    out=alpha_raw[:, j : j + 1],
                    in_=w_tiles[j],
                    axis=mybir.AxisListType.X,
                    op=Alu.add,
                    apply_absolute_value=True,
                )

            # ---- quant prologue: first few s-tiles' quant+transpose, so the
            # PE has ready work while the weight ternarization chain resolves.
            x_pairs = {}

            def emit_quant(s):
                # x rows arrive two s-tiles per 1 MiB DMA (better DMA efficiency)
                if s % 2 == 0:
                    x2 = xp.tile([P, 2, IN_F], f32, tag="x")
                    if s == 0:
                        # two single-tile DMAs: the s=0 chain starts as soon
                        # as the first 512 KiB lands (subtile deps)
                        for g in range(2):
                            nc.sync.dma_start(
                                out=x2[:, g, :],
                                in_=x_in[(s + g) * P : (s + g + 1) * P, :],
                            )
                    else:
                        nc.sync.dma_start(
                            out=x2,
                            in_=x_in[s * P : (s + 2) * P, :].rearrange(
                                "(two p) f -> p two f", p=P
                            ),
                        )
                    x_pairs[s] = x2
                    x_t = x2[:, 0, :]
                else:
                    x_t = x_pairs.pop(s - 1)[:, 1, :]
                ma = stats.tile([P, 1], f32, tag="ma")
                nc.vector.tensor_reduce(
                    out=ma,
                    in_=x_t,
                    axis=mybir.AxisListType.X,
                    op=Alu.max,
                    apply_absolute_value=True,
                )
                # row scale = max(ma, EPS)/7 ; inv = 7/max(ma, EPS)
                ma7 = ma7p.tile([P, 1], f32, tag="ma7")
                nc.vector.tensor_scalar(
                    out=ma7,
                    in0=ma,
                    scalar1=float(1.0 / 7.0),
                    scalar2=float(EPS / 7.0),
                    op0=Alu.mult,
                    op1=Alu.max,
                )
                inv = stats.tile([P, 1], f32, tag="inv")
                nc.vector.reciprocal(out=inv, in_=ma7)
                # t = x*inv + MAGIC (f32; fraction now rounded half-to-even)
                t_t = tp.tile([P, IN_F], f32, tag="t")
                nc.gpsimd.tensor_scalar(
                    out=t_t,
                    in0=x_t,
                    scalar1=inv,
                    scalar2=MAGIC,
                    op0=Alu.mult,
                    op1=Alu.add,
                )
                # transpose t into [i, s] layout via PE (8 blocks, one psum tile)
                qt_ps = tpsum.tile([P, IN_F], f32, tag="tps")
                for k in range(N_KTILES):
                    nc.tensor.transpose(
                        qt_ps[:, k * P : (k + 1) * P],
                        t_t[:, k * P : (k + 1) * P],
                        identity_f,
                    )
                # evict with fused -MAGIC subtract + fp8 cast (exact ints)
                qt_sb = qtp.tile([P, N_KTILES, P], f8, tag="qt")
                nc.scalar.activation(
                    out=qt_sb.rearrange("p k c -> p (k c)"),
                    in_=qt_ps,
                    func=Act.Identity,
                    bias=magneg,
                    scale=1.0,
                )
                return ma7, qt_sb

            out_pairs = {}

            def emit_matmul(s, ma7, qt_sb):
                # output rows leave two s-tiles per 1 MiB DMA
                if s % 2 == 0:
                    out2 = outp.tile([P, 2, OUT_F], f32, tag="o")
                    out_pairs[s] = out2
                    out_sb = out2[:, 0, :]
                else:
                    out2 = out_pairs[s - 1]
                    out_sb = out2[:, 1, :]
                ps = mpsum.tile([P, OUT_F], f32, tag="mm")
                for h in range(N_OHALF):
                    for t in range(N_KTILES // 2):
                        nc.tensor.matmul(
                            ps[:, h * MM_N : (h + 1) * MM_N],
                            lhsT=qt_sb[:, 2 * t : 2 * t + 2, :],
                            rhs=signT8[
                                :, 2 * t : 2 * t + 2, h * MM_N : (h + 1) * MM_N
                            ],
                            start=(t == 0),
                            stop=(t == N_KTILES // 2 - 1),
                            perf_mode=mybir.MatmulPerfMode.DoubleRow,
                        )
                # out = (S * rowscale) * colscale  (one fused PSUM eviction)
                nc.vector.scalar_tensor_tensor(
                    out=out_sb,
                    in0=ps,
                    scalar=ma7,
                    in1=colb,
                    op0=Alu.mult,
                    op1=Alu.mult,
                )
                if with_bias:
                    nc.gpsimd.tensor_tensor(
                        out=out_sb, in0=out_sb, in1=biasb, op=Alu.add
                    )
                if s % 2 == 1:
                    nc.scalar.dma_start(
                        out=out_d[(s - 1) * P : (s + 1) * P, :].rearrange(
                            "(two p) f -> p two f", p=P
                        ),
                        in_=out_pairs.pop(s - 1),
                    )

            def emit_wprep_tail():
                # global threshold = 0.05 * mean(|w|)
                g0 = stats.tile([P, 1], f32, tag="g0")
                nc.vector.tensor_reduce(
                    out=g0, in_=alpha_raw, axis=mybir.AxisListType.X, op=Alu.add
                )
                g1 = stats.tile([P, 1], f32, tag="g1")
                nc.gpsimd.partition_all_reduce(
                    out_ap=g1, in_ap=g0, channels=P, reduce_op=bass_isa.ReduceOp.add
                )
                nc.vector.tensor_scalar(
                    out=thr,
                    in0=g1,
                    scalar1=float(0.05 / (OUT_F * IN_F)),
                    scalar2=None,
                    op0=Alu.mult,
                )
                nc.vector.tensor_scalar(
                    out=nthr, in0=thr, scalar1=-1.0, scalar2=None, op0=Alu.mult
                )
                # alpha[o] = rowsum / IN_F
                nc.vector.tensor_scalar(
                    out=alpha_sb,
                    in0=alpha_raw,
                    scalar1=float(1.0 / IN_F),
                    scalar2=None,
                    op0=Alu.mult,
                )

                # ternary sign: sign = (w >= thr) + (w > -thr) - 1, entirely
                # on GPSIMD (three 1/2-input ops) to keep DVE free
                for j in range(N_OTILES):
                    tmp = wtmp.tile([P, IN_F], f32, tag="tmp")
                    nc.gpsimd.tensor_scalar(
                        out=tmp,
                        in0=w_tiles[j],
                        scalar1=nthr,
                        scalar2=-1.0,
                        op0=Alu.is_gt,
                        op1=Alu.add,
                    )
                    sgn = signp.tile([P, IN_F], bf16, tag="sgn")
                    nc.vector.scalar_tensor_tensor(
                        out=sgn,
                        in0=w_tiles[j],
                        scalar=thr,
                        in1=tmp,
                        op0=Alu.is_ge,
                        op1=Alu.add,
                    )
                    # transpose 8x [128,128] blocks into one PSUM bank, evict
                    ps = tpsum.tile([P, IN_F], bf16, tag="tps")
                    for k in range(N_KTILES):
                        nc.tensor.transpose(
                            ps[:, k * P : (k + 1) * P],
                            sgn[:, k * P : (k + 1) * P],
                            identity,
                        )
                    nc.scalar.activation(
                        out=signT8[:, :, j * P : (j + 1) * P],
                        in_=ps.rearrange("p (k c) -> p k c", k=N_KTILES),
                        func=Act.Copy,
                    )

                # column scale alpha broadcast to all partitions via DRAM bounce
                nc.sync.dma_start(
                    out=alpha_dram.rearrange("j p -> p j"), in_=alpha_sb
                )
                alpha_flat = alpha_dram.rearrange("j p -> (j p)")
                bcast_src = bass.AP(
                    tensor=alpha_flat.tensor,
                    offset=alpha_flat.offset,
                    ap=[[0, P]] + list(alpha_flat.ap),
                )
                nc.sync.dma_start(out=colb, in_=bcast_src)

                if with_bias:
                    bias_src = bass.AP(
                        tensor=b_in.tensor
                        if hasattr(b_in, "tensor")
                        else b_in[:].tensor,
                        offset=b_in[:].offset,
                        ap=[[0, P]] + list(b_in[:].ap),
                    )
                    nc.sync.dma_start(out=biasb, in_=bias_src)

            thr = singles.tile([P, 1], f32)
            nthr = singles.tile([P, 1], f32)
            alpha_sb = singles.tile([P, N_OTILES], f32)
            alpha_dram = dramp.tile([N_OTILES, P], f32)
            colb = singles.tile([P, OUT_F], f32)
            biasb = None
            if with_bias:
                biasb = singles.tile([P, OUT_F], f32, tag="biasb")

            # Phase 1: quantize + transpose ALL s-tiles (PE does transposes
            # while the weight-ternarization chain resolves); |w| row-sums
            # interleave into the ACT stream between the early evictions, and
            # the full sign chain is emitted early (after s=4) so it sits near
            # the front of each engine's FIFO.
            LEAD = min(int(os.environ.get("KLEAD", "6")), N_STILES)
            # |w| row-sum pairs finish by s=3, the sign chain is emitted at
            # WPREP_S, and the first matmul emission is clamped to come after
            # it: a matmul emitted before the signT8 writes would read the
            # uninitialized tile (Tile deps follow program order).
            WPREP_S = min(N_OTILES // 2, N_STILES - 1)
            LEAD = max(LEAD, WPREP_S + 1)
            prologue = []
            for s in range(N_STILES):
                prologue.append(emit_quant(s))
                if s == 0:
                    emit_wload_even((0, 2) if N_STILES > 1 else (0, 2, 4, 6))
                elif s == 1:
                    emit_wload_even((4, 6))
                for j in (2 * s, 2 * s + 1):
                    if j < N_OTILES:
                        emit_wabs(j)
                if s == N_STILES - 1 and 2 * N_STILES < N_OTILES:
                    for j in range(2 * N_STILES, N_OTILES):
                        emit_wabs(j)
                if s == WPREP_S:
                    emit_wprep_tail()
                    w_tiles.clear()
                    _wstack.close()  # releases the 32KB weight pool
                if s >= LEAD:
                    emit_matmul(s - LEAD, *prologue[s - LEAD])
            for s in range(max(0, N_STILES - LEAD), N_STILES):
                emit_matmul(s, *prologue[s])

    nc.compile()
    return nc


def _get_program(with_bias: bool):
    key = bool(with_bias)
    if key not in _prog_cache:
        _prog_cache[key] = _build_program(key)
    return _prog_cache[key]


def kernel(x: np.ndarray, weight: np.ndarray, bias: np.ndarray) -> np.ndarray:
    from concourse.bass_utils import run_bass_kernel_spmd

    B, S, in_f = x.shape
    out_f = weight.shape[0]
    assert in_f == IN_F and out_f == OUT_F and B * S == N_CORES * S_SHARD

    xf = np.ascontiguousarray(x.astype(np.float32, copy=False).reshape(-1, IN_F))
    w = np.ascontiguousarray(weight.astype(np.float32, copy=False))
    b = np.ascontiguousarray(bias.astype(np.float32, copy=False))

    with_bias = bool(np.any(b != 0.0))
    nc = _get_program(with_bias)

    in_maps = []
    for c in range(N_CORES):
        m = {
            "x_shard": xf[c * S_SHARD : (c + 1) * S_SHARD],
            "weight": w,
        }
        if with_bias:
            m["bias"] = b
        in_maps.append(m)

    res = run_bass_kernel_spmd(nc, in_maps, core_ids=list(range(N_CORES)))
    out = np.concatenate([res.results[c]["out"] for c in range(N_CORES)], axis=0)
    return out.reshape(B, S, OUT_F).astype(np.float32, copy=False)

